# revision 17
# baseline (speedup 1.0000x reference)
"""Factored TRN2 kernel: out = x @ M with M = W.T @ cos_basis, then /max.

Each core computes M[:, s*256:(s+1)*256] (its 256-column shard of M, no
redundancy) and then out.T[s-shard, :] = (x @ M[:, shard]).T for the FULL
batch. Column sharding means x is replicated (16MB bf16 per core) but total
PE work drops 28% vs the direct two-GEMM form: phase 1 is 272 MM-256 pairs
(1.14e9 MAC) instead of 272 MM-512, phase 2 is 256 MM-512 (2.15e9 MAC).

Phase 1 (M-form): psum[l-tile, 256] += W-block[f,l].T @ cos_shard[f, 256]
  lhsT = W blocks with partition=f (host: Wp[li, p(f), fi... see prep)
  -> M tiles [128 l, 256 l2] in SBUF bf16: exactly the phase-2 lhsT layout.
Phase 2: psum[l2-part, 512 m] += M-block[l, l2].T @ xT[l, m]
  out.T tiles written per (l2p, m-chunk); host transposes/concats (free).

Max: same two-stage scalar AllReduce(max) as the direct kernel.
"""
import numpy as np
import ml_dtypes

import concourse.bass as bass
import concourse.bass_isa as bass_isa
import concourse.bacc as bacc
import concourse.mybir as mybir
import concourse.tile as tile
import concourse.bass_utils as bass_utils

N_CORES = 8
B, L, F = 4096, 2048, 2074
FP = 2176               # F padded to 17 full 128-tiles
CS = L // N_CORES       # 256 M-columns per core
LT = L // 128           # 16 l-tiles
FT = FP // 128          # 17 f-tiles
MC = 4                  # m-chunks of 1024 in phase 2
MW = B // MC            # 1024 batch columns per chunk
F32 = mybir.dt.float32
BF16 = mybir.dt.bfloat16
NP_BF16 = ml_dtypes.bfloat16


def _emit(nc, tc, Wl, cs, xTf, outT):
    with (
        tc.tile_pool(name="wp", bufs=6) as wp,
        tc.tile_pool(name="csp", bufs=1) as csp,
        tc.tile_pool(name="mp", bufs=1) as mp,
        tc.tile_pool(name="xp", bufs=2) as xp,
        tc.tile_pool(name="op", bufs=1) as op,
        tc.tile_pool(name="sp", bufs=1) as sp,
        tc.tile_pool(name="ps1", bufs=2, space="PSUM") as ps1,
        tc.tile_pool(name="ps2", bufs=3, space="PSUM") as ps2,
        tc.tile_pool(name="dp", bufs=1, space="DRAM") as dp,
    ):
        def armax(lmx, tag):
            lmb = sp.tile([128, 1], F32, name=f"lmb_{tag}")
            nc.gpsimd.partition_all_reduce(lmb[:], lmx[:], channels=128,
                                           reduce_op=bass_isa.ReduceOp.max)
            cc_in = dp.tile([1], F32, name=f"ccin_{tag}")
            cc_out = dp.tile([1], F32, name=f"ccout_{tag}")
            nc.gpsimd.dma_start(cc_in[:], lmb[0:1, 0])
            nc.gpsimd.collective_compute(
                "AllReduce", mybir.AluOpType.max,
                replica_groups=[list(range(N_CORES))],
                ins=[cc_in[:]], outs=[cc_out[:]])
            return cc_out

        # ---- phase 1: M[:, shard] = W.T @ cos[:, shard] ----
        # cs arrives in 4 fi-range pieces spread over the queues so the
        # first matmul waits only on piece 0 + wl0 (~1MB) instead of 1.6MB.
        cst = csp.tile([128, FT * CS], BF16, name="cst")
        cstv = cst[:].rearrange("p (fi n) -> p fi n", fi=FT)
        wl = [wp.tile([128, FT * 128], BF16, tag="w", name=f"w{li % 6}")
              for li in range(LT)]

        def wdma(li, q):
            q.dma_start(wl[li][:].rearrange("p (fi b) -> p fi b", fi=FT),
                        Wl[li])

        nc.scalar.dma_start(cstv[:, 0:5], cs[:, 0:5])
        wdma(0, nc.sync)
        nc.gpsimd.dma_start(cstv[:, 5:11], cs[:, 5:11])
        nc.gpsimd.dma_start(cstv[:, 11:FT], cs[:, 11:FT])
        wdma(1, nc.scalar)
        wdma(2, nc.sync)
        wdma(3, nc.scalar)
        wdma(4, nc.sync)
        mt = [mp.tile([128, CS], BF16, name=f"m{li}") for li in range(LT)]
        joiner = dp.tile([1, 1], BF16, name="joiner")
        for li in range(LT):
            # prefetch depth 5 = bufs-1: wl[li+5] shares a buffer with
            # wl[li-1], whose readers are already emitted, so the write is
            # ordered correctly. (Depth bufs would overwrite wl[li] before
            # its later-emitted matmuls read it.) The deep prefetch banks
            # W columns during the fast pre-barrier DMA window to ride out
            # the 8-core HBM-contention crawl at ~21-40us.
            if li + 5 < LT:
                wdma(li + 5, nc.scalar if li % 2 == 0 else nc.sync)
            ps = ps1.tile([128, CS], F32, tag="m")
            for fi in range(FT):
                nc.tensor.matmul(ps[:], wl[li][:, fi * 128:(fi + 1) * 128],
                                 cst[:, fi * CS:(fi + 1) * CS],
                                 start=(fi == 0), stop=(fi == FT - 1))
            nc.vector.tensor_copy(mt[li][:], ps[:])

        # ---- phase 2: out.T[l2p, m] += M-block.T @ xT, streamed per
        # 1024-column m-chunk (4MB bf16 each, 2KB lines) ----
        # The x stream rides the HWDGE queues (engine-side issue ~600ns; a
        # 4MB gpsimd SWDGE issue occupies the engine ~35us, which would
        # queue the stage-1 collective trigger behind it until ~105us).
        # Joiners reading mid-phase-1 M tiles delay each stream start until
        # the W stream no longer needs the bandwidth.
        xc = [xp.tile([128, LT * MW], BF16, tag="x", name=f"xc{c % 2}")
              for c in range(MC)]
        joiner2 = dp.tile([1, 1], BF16, name="joiner2")
        nc.sync.dma_start(joiner[:], mt[6][0:1, 0:1])
        nc.sync.dma_start(xc[0][:].rearrange("p (li m) -> p li m", li=LT),
                          xTf[0])
        nc.scalar.dma_start(joiner2[:], mt[9][0:1, 0:1])
        nc.scalar.dma_start(xc[1][:].rearrange("p (li m) -> p li m", li=LT),
                            xTf[1])

        otp = [op.tile([128, B], BF16, name=f"otp{l2p}") for l2p in range(2)]
        vmaxes = sp.tile([128, 2], F32)
        lm = sp.tile([128, 1], F32)
        lm2 = sp.tile([128, 1], F32)
        for mc in range(MC):
            xcur = xc[mc % 2]
            psts = []
            for l2p in range(2):
                pst = ps2.tile([128, MW], F32, tag="o",
                               name=f"ps2_{mc}_{l2p}")
                psts.append(pst)
                for li in range(LT):
                    nc.tensor.matmul(
                        pst[:, 0:512],
                        mt[li][:, l2p * 128:(l2p + 1) * 128],
                        xcur[:, li * MW:li * MW + 512],
                        start=(li == 0), stop=(li == LT - 1))
                for li in range(LT):
                    nc.tensor.matmul(
                        pst[:, 512:MW],
                        mt[li][:, l2p * 128:(l2p + 1) * 128],
                        xcur[:, li * MW + 512:(li + 1) * MW],
                        start=(li == 0), stop=(li == LT - 1))
                if l2p == 1 and mc + 2 < MC:
                    (nc.sync if mc == 0 else nc.scalar).dma_start(
                        xc[mc % 2][:].rearrange("p (li m) -> p li m", li=LT),
                        xTf[mc + 2])
                # running max first (the AllReduce trigger chain), PSUM->
                # SBUF copies after, so the stage-2 trigger is not queued
                # behind a 1.1us copy on the vector engine. Stage 1 covers
                # only (mc0, l2p0) so its trigger fires ~1/8 into phase 2:
                # maximum margin to hide the collective (and the noisy
                # prelude barrier) under the remaining compute.
                nc.vector.reduce_max(vmaxes[:, l2p:l2p + 1], pst[:],
                                     axis=mybir.AxisListType.X)
                if mc == 0 and l2p == 0:
                    nc.vector.tensor_copy(lm[:], vmaxes[:, 0:1])
                    cc1_out = armax(lm, "s1")
                elif mc == 0 and l2p == 1:
                    nc.vector.tensor_copy(lm2[:], vmaxes[:, 1:2])
                else:
                    nc.vector.tensor_scalar_max(lm2[:], lm2[:],
                                                vmaxes[:, l2p:l2p + 1])
            for l2p in range(2):
                nc.vector.tensor_copy(
                    otp[l2p][:, mc * MW:(mc + 1) * MW], psts[l2p][:])

        cc2_out = armax(lm2, "s2")
        gbc1 = sp.tile([128, 1], F32)
        nc.sync.dma_start(gbc1[:], cc1_out[:].partition_broadcast(128))
        gbc2 = sp.tile([128, 1], F32)
        nc.scalar.dma_start(gbc2[:], cc2_out[:].partition_broadcast(128))
        gbc = sp.tile([128, 1], F32)
        nc.vector.tensor_scalar_max(gbc[:], gbc1[:], gbc2[:, 0:1])
        rbc = sp.tile([128, 1], F32)
        nc.vector.reciprocal(rbc[:], gbc[:])

        # scale + store in 1024-column quarters: first write DMA starts
        # ~0.5us after the reciprocal, pipelined across both HWDGE queues.
        for qi in range(MC):
            for l2p in range(2):
                sl = slice(qi * MW, (qi + 1) * MW)
                nc.vector.tensor_scalar_mul(otp[l2p][:, sl],
                                            otp[l2p][:, sl], rbc[:, 0:1])
                (nc.sync if (qi * 2 + l2p) % 2 == 0 else nc.scalar).dma_start(
                    outT[l2p][:, sl], otp[l2p][:, sl])


def _build():
    nc = bacc.Bacc("TRN2", target_bir_lowering=False, debug=False,
                   num_devices=N_CORES)
    # Wl[li, p(f in tile), fi, b(l in tile)] = Wp[fi*128+p, li*128+b]
    Wl = nc.dram_tensor("Wl", [LT, 128, FT, 128], BF16, kind="ExternalInput")
    # cs[p(f in tile), fi, n] = cosp[fi*128+p, shard_cols[n]]
    cs = nc.dram_tensor("cs", [128, FT, CS], BF16, kind="ExternalInput")
    # xTf[mc, p(l in tile), li, m] = x[mc*1024+m, li*128+p]
    xTf = nc.dram_tensor("xTf", [MC, 128, LT, MW], BF16,
                         kind="ExternalInput")
    outT = nc.dram_tensor("outT", [2, 128, B], BF16, kind="ExternalOutput")
    with tile.TileContext(nc) as tc:
        _emit(nc, tc, Wl, cs, xTf, outT)
    nc.compile()
    return nc


_cached_nc = None


def _get_nc():
    global _cached_nc
    if _cached_nc is None:
        _cached_nc = _build()
    return _cached_nc


def _prep_inputs(x, W, cos_basis):
    x = np.ascontiguousarray(x, dtype=np.float32)
    W = np.ascontiguousarray(W, dtype=np.float32)
    cos = np.ascontiguousarray(cos_basis, dtype=np.float32)
    Wp = np.zeros((FP, L), dtype=np.float32)
    Wp[:F] = W
    cosp = np.zeros((FP, L), dtype=np.float32)
    cosp[:F] = cos
    # Wl[li, p, fi, b] = Wp[fi*128+p, li*128+b]
    Wl = np.ascontiguousarray(
        Wp.reshape(FT, 128, LT, 128).transpose(2, 1, 0, 3).astype(NP_BF16))
    # per-core cos shard: cs[p, fi, n] = cosp[fi*128+p, s*CS+n]
    csr = cosp.reshape(FT, 128, N_CORES, CS).transpose(2, 1, 0, 3)
    css = [np.ascontiguousarray(csr[s].astype(NP_BF16))
           for s in range(N_CORES)]
    # xTf[mc, p, li, m] = x[mc*MW+m, li*128+p]  (replicated to all cores)
    xTf = np.ascontiguousarray(
        x.reshape(MC, MW, LT, 128).transpose(0, 3, 2, 1).astype(NP_BF16))
    return Wl, css, xTf


_out_names = ["outT"]


def _in_maps(x, W, cos_basis):
    Wl, css, xTf = _prep_inputs(x, W, cos_basis)
    return [{"Wl": Wl, "cs": css[i], "xTf": xTf} for i in range(N_CORES)]


def _post(results):
    # outT[core s][l2p, p, m] = out[m, s*CS + l2p*128 + p]
    shards = []
    for i in range(N_CORES):
        o = results[i]["outT"].astype(np.float32)  # [2, 128, B]
        shards.append(o.reshape(CS, B).T)          # [B, CS]
    return np.ascontiguousarray(np.concatenate(shards, axis=1))


def kernel(x, W, cos_basis, _trace=False, _trace_kwargs=None):
    in_maps = _in_maps(x, W, cos_basis)
    nc = _get_nc()
    res = bass_utils.run_bass_kernel_spmd(
        nc, in_maps, core_ids=list(range(N_CORES)), trace=_trace,
        **(_trace_kwargs or {}))
    out = _post(res.results)
    if _trace:
        kernel.last_result = res
    return out


# revision 18
# speedup vs baseline: 1.3096x; 1.3096x over previous
"""Factored TRN2 kernel: out = x @ M with M = W.T @ cos_basis, then /max.

Each core computes M[:, s*256:(s+1)*256] (its 256-column shard of M, no
redundancy) and then out.T[s-shard, :] = (x @ M[:, shard]).T for the FULL
batch. Column sharding means x is replicated (16MB bf16 per core) but total
PE work drops 28% vs the direct two-GEMM form: phase 1 is 272 MM-256 pairs
(1.14e9 MAC) instead of 272 MM-512, phase 2 is 256 MM-512 (2.15e9 MAC).

Phase 1 (M-form): psum[l-tile, 256] += W-block[f,l].T @ cos_shard[f, 256]
  lhsT = W blocks with partition=f (host: Wp[li, p(f), fi... see prep)
  -> M tiles [128 l, 256 l2] in SBUF bf16: exactly the phase-2 lhsT layout.
Phase 2: psum[l2-part, 512 m] += M-block[l, l2].T @ xT[l, m]
  out.T tiles written per (l2p, m-chunk); host transposes/concats (free).

Max: same two-stage scalar AllReduce(max) as the direct kernel.
"""
import numpy as np
import ml_dtypes

import concourse.bass as bass
import concourse.bass_isa as bass_isa
import concourse.bacc as bacc
import concourse.mybir as mybir
import concourse.tile as tile
import concourse.bass_utils as bass_utils

N_CORES = 8
B, L, F = 4096, 2048, 2074
FP = 2176               # F padded to 17 full 128-tiles
CS = L // N_CORES       # 256 M-columns per core
LT = L // 128           # 16 l-tiles
FT = FP // 128          # 17 f-tiles
MC = 4                  # m-chunks of 1024 in phase 2
MW = B // MC            # 1024 batch columns per chunk
F32 = mybir.dt.float32
BF16 = mybir.dt.bfloat16
NP_BF16 = ml_dtypes.bfloat16


def _emit(nc, tc, Wl, cs, xTf, outT):
    with (
        tc.tile_pool(name="wp", bufs=6) as wp,
        tc.tile_pool(name="csp", bufs=1) as csp,
        tc.tile_pool(name="mp", bufs=1) as mp,
        tc.tile_pool(name="xp", bufs=2) as xp,
        tc.tile_pool(name="op", bufs=1) as op,
        tc.tile_pool(name="sp", bufs=1) as sp,
        tc.tile_pool(name="ps1", bufs=2, space="PSUM") as ps1,
        tc.tile_pool(name="ps2", bufs=3, space="PSUM") as ps2,
        tc.tile_pool(name="dp", bufs=1, space="DRAM") as dp,
    ):
        def armax(lmx, tag):
            lmb = sp.tile([128, 1], F32, name=f"lmb_{tag}")
            nc.gpsimd.partition_all_reduce(lmb[:], lmx[:], channels=128,
                                           reduce_op=bass_isa.ReduceOp.max)
            cc_in = dp.tile([1], F32, name=f"ccin_{tag}")
            cc_out = dp.tile([1], F32, name=f"ccout_{tag}")
            nc.gpsimd.dma_start(cc_in[:], lmb[0:1, 0])
            nc.gpsimd.collective_compute(
                "AllReduce", mybir.AluOpType.max,
                replica_groups=[list(range(N_CORES))],
                ins=[cc_in[:]], outs=[cc_out[:]])
            return cc_out

        # ---- phase 1: M[:, shard] = W.T @ cos[:, shard] ----
        # cs arrives in 4 fi-range pieces spread over the queues so the
        # first matmul waits only on piece 0 + wl0 (~1MB) instead of 1.6MB.
        cst = csp.tile([128, FT * CS], BF16, name="cst")
        cstv = cst[:].rearrange("p (fi n) -> p fi n", fi=FT)
        wl = [wp.tile([128, FT * 128], BF16, tag="w", name=f"w{li % 6}")
              for li in range(LT)]

        def wdma(li, q):
            q.dma_start(wl[li][:].rearrange("p (fi b) -> p fi b", fi=FT),
                        Wl[li])

        nc.scalar.dma_start(cstv[:, 0:5], cs[:, 0:5])
        wdma(0, nc.sync)
        nc.gpsimd.dma_start(cstv[:, 5:11], cs[:, 5:11])
        nc.gpsimd.dma_start(cstv[:, 11:FT], cs[:, 11:FT])
        wdma(1, nc.scalar)
        wdma(2, nc.sync)
        wdma(3, nc.scalar)
        wdma(4, nc.sync)
        mt = [mp.tile([128, CS], BF16, name=f"m{li}") for li in range(LT)]
        joiner = dp.tile([1, 1], BF16, name="joiner")
        for li in range(LT):
            # prefetch depth 5 = bufs-1: wl[li+5] shares a buffer with
            # wl[li-1], whose readers are already emitted, so the write is
            # ordered correctly. (Depth bufs would overwrite wl[li] before
            # its later-emitted matmuls read it.) The deep prefetch banks
            # W columns during the fast pre-barrier DMA window to ride out
            # the 8-core HBM-contention crawl at ~21-40us.
            if li + 5 < LT:
                wdma(li + 5, nc.scalar if li % 2 == 0 else nc.sync)
            ps = ps1.tile([128, CS], F32, tag="m")
            for fi in range(FT):
                nc.tensor.matmul(ps[:], wl[li][:, fi * 128:(fi + 1) * 128],
                                 cst[:, fi * CS:(fi + 1) * CS],
                                 start=(fi == 0), stop=(fi == FT - 1))
            nc.vector.tensor_copy(mt[li][:], ps[:])

        # ---- phase 2: out.T[l2p, m] += M-block.T @ xT, streamed per
        # 1024-column m-chunk (4MB bf16 each, 2KB lines) ----
        # The x stream rides the HWDGE queues (engine-side issue ~600ns; a
        # 4MB gpsimd SWDGE issue occupies the engine ~35us, which would
        # queue the stage-1 collective trigger behind it until ~105us).
        # Joiners reading mid-phase-1 M tiles delay each stream start until
        # the W stream no longer needs the bandwidth.
        xc = [xp.tile([128, LT * MW], BF16, tag="x", name=f"xc{c % 2}")
              for c in range(MC)]
        joiner2 = dp.tile([1, 1], BF16, name="joiner2")
        nc.sync.dma_start(joiner[:], mt[6][0:1, 0:1])
        nc.sync.dma_start(xc[0][:].rearrange("p (li m) -> p li m", li=LT),
                          xTf[0])
        nc.scalar.dma_start(joiner2[:], mt[9][0:1, 0:1])
        nc.scalar.dma_start(xc[1][:].rearrange("p (li m) -> p li m", li=LT),
                            xTf[1])

        otp = [op.tile([128, B], BF16, name=f"otp{l2p}") for l2p in range(2)]
        vmaxes = sp.tile([128, 2], F32)
        lm = sp.tile([128, 1], F32)
        lm2 = sp.tile([128, 1], F32)
        for mc in range(MC):
            xcur = xc[mc % 2]
            psts = []
            for l2p in range(2):
                pst = ps2.tile([128, MW], F32, tag="o",
                               name=f"ps2_{mc}_{l2p}")
                psts.append(pst)
                # the two 512-wide halves interleaved per li: consecutive
                # matmuls share the same lhsT M-block, so each weight load
                # serves two matmuls (and the halves live in different PSUM
                # banks, so their accumulate chains stay independent).
                for li in range(LT):
                    nc.tensor.matmul(
                        pst[:, 0:512],
                        mt[li][:, l2p * 128:(l2p + 1) * 128],
                        xcur[:, li * MW:li * MW + 512],
                        start=(li == 0), stop=(li == LT - 1))
                    nc.tensor.matmul(
                        pst[:, 512:MW],
                        mt[li][:, l2p * 128:(l2p + 1) * 128],
                        xcur[:, li * MW + 512:(li + 1) * MW],
                        start=(li == 0), stop=(li == LT - 1))
                if l2p == 1 and mc + 2 < MC:
                    (nc.sync if mc == 0 else nc.scalar).dma_start(
                        xc[mc % 2][:].rearrange("p (li m) -> p li m", li=LT),
                        xTf[mc + 2])
                # running max first (the AllReduce trigger chain), PSUM->
                # SBUF copies after, so the stage-2 trigger is not queued
                # behind a 1.1us copy on the vector engine. Stage 1 covers
                # only (mc0, l2p0) so its trigger fires ~1/8 into phase 2:
                # maximum margin to hide the collective (and the noisy
                # prelude barrier) under the remaining compute.
                nc.vector.reduce_max(vmaxes[:, l2p:l2p + 1], pst[:],
                                     axis=mybir.AxisListType.X)
                if mc == 0 and l2p == 0:
                    nc.vector.tensor_copy(lm[:], vmaxes[:, 0:1])
                    cc1_out = armax(lm, "s1")
                elif mc == 0 and l2p == 1:
                    nc.vector.tensor_copy(lm2[:], vmaxes[:, 1:2])
                else:
                    nc.vector.tensor_scalar_max(lm2[:], lm2[:],
                                                vmaxes[:, l2p:l2p + 1])
            for l2p in range(2):
                nc.vector.tensor_copy(
                    otp[l2p][:, mc * MW:(mc + 1) * MW], psts[l2p][:])

        cc2_out = armax(lm2, "s2")
        gbc1 = sp.tile([128, 1], F32)
        nc.sync.dma_start(gbc1[:], cc1_out[:].partition_broadcast(128))
        gbc2 = sp.tile([128, 1], F32)
        nc.scalar.dma_start(gbc2[:], cc2_out[:].partition_broadcast(128))
        gbc = sp.tile([128, 1], F32)
        nc.vector.tensor_scalar_max(gbc[:], gbc1[:], gbc2[:, 0:1])
        rbc = sp.tile([128, 1], F32)
        nc.vector.reciprocal(rbc[:], gbc[:])

        # scale + store in 1024-column quarters on the vector engine only
        # (gpsimd elementwise ops are ~15us per quarter - never again);
        # write DMAs pipeline across both HWDGE queues.
        for qi in range(MC):
            for l2p in range(2):
                sl = slice(qi * MW, (qi + 1) * MW)
                nc.vector.tensor_scalar_mul(otp[l2p][:, sl],
                                            otp[l2p][:, sl], rbc[:, 0:1])
                (nc.sync if l2p == 0 else nc.scalar).dma_start(
                    outT[l2p][:, sl], otp[l2p][:, sl])


def _build():
    nc = bacc.Bacc("TRN2", target_bir_lowering=False, debug=False,
                   num_devices=N_CORES)
    # Wl[li, p(f in tile), fi, b(l in tile)] = Wp[fi*128+p, li*128+b]
    Wl = nc.dram_tensor("Wl", [LT, 128, FT, 128], BF16, kind="ExternalInput")
    # cs[p(f in tile), fi, n] = cosp[fi*128+p, shard_cols[n]]
    cs = nc.dram_tensor("cs", [128, FT, CS], BF16, kind="ExternalInput")
    # xTf[mc, p(l in tile), li, m] = x[mc*1024+m, li*128+p]
    xTf = nc.dram_tensor("xTf", [MC, 128, LT, MW], BF16,
                         kind="ExternalInput")
    outT = nc.dram_tensor("outT", [2, 128, B], BF16, kind="ExternalOutput")
    with tile.TileContext(nc) as tc:
        _emit(nc, tc, Wl, cs, xTf, outT)
    nc.compile()
    return nc


_cached_nc = None


def _get_nc():
    global _cached_nc
    if _cached_nc is None:
        _cached_nc = _build()
    return _cached_nc


def _prep_inputs(x, W, cos_basis):
    x = np.ascontiguousarray(x, dtype=np.float32)
    W = np.ascontiguousarray(W, dtype=np.float32)
    cos = np.ascontiguousarray(cos_basis, dtype=np.float32)
    Wp = np.zeros((FP, L), dtype=np.float32)
    Wp[:F] = W
    cosp = np.zeros((FP, L), dtype=np.float32)
    cosp[:F] = cos
    # Wl[li, p, fi, b] = Wp[fi*128+p, li*128+b]
    Wl = np.ascontiguousarray(
        Wp.reshape(FT, 128, LT, 128).transpose(2, 1, 0, 3).astype(NP_BF16))
    # per-core cos shard: cs[p, fi, n] = cosp[fi*128+p, s*CS+n]
    csr = cosp.reshape(FT, 128, N_CORES, CS).transpose(2, 1, 0, 3)
    css = [np.ascontiguousarray(csr[s].astype(NP_BF16))
           for s in range(N_CORES)]
    # xTf[mc, p, li, m] = x[mc*MW+m, li*128+p]  (replicated to all cores)
    xTf = np.ascontiguousarray(
        x.reshape(MC, MW, LT, 128).transpose(0, 3, 2, 1).astype(NP_BF16))
    return Wl, css, xTf


_out_names = ["outT"]


def _in_maps(x, W, cos_basis):
    Wl, css, xTf = _prep_inputs(x, W, cos_basis)
    return [{"Wl": Wl, "cs": css[i], "xTf": xTf} for i in range(N_CORES)]


def _post(results):
    # outT[core s][l2p, p, m] = out[m, s*CS + l2p*128 + p]
    shards = []
    for i in range(N_CORES):
        o = results[i]["outT"].astype(np.float32)  # [2, 128, B]
        shards.append(o.reshape(CS, B).T)          # [B, CS]
    return np.ascontiguousarray(np.concatenate(shards, axis=1))


def kernel(x, W, cos_basis, _trace=False, _trace_kwargs=None):
    in_maps = _in_maps(x, W, cos_basis)
    nc = _get_nc()
    res = bass_utils.run_bass_kernel_spmd(
        nc, in_maps, core_ids=list(range(N_CORES)), trace=_trace,
        **(_trace_kwargs or {}))
    out = _post(res.results)
    if _trace:
        kernel.last_result = res
    return out
